# revision 34
# baseline (speedup 1.0000x reference)
"""Causal self-attention (B=4, T=2048, C=2048, H=16) on 8 TRN2 NeuronCores.

Sharding: core c owns batch b=c//2 and a balanced set of query stripes
(parity s=c%2). KV projection is split by head between the two cores of a
batch (even core: heads 0-7, odd core: heads 8-15) and exchanged with a
pairwise AllGather — rank order makes gathered head-slot order identical on
both cores, so the SPMD graph stays symmetric. Attention + output projection
run on each core's 1024 query rows; the host gathers the row-shards.

Layouts (all transposed so no on-chip transposes are ever needed):
  xT [C, T] (host-transposed); qT/kT [C_head, T*] head-major with
  rope-paired-permuted d (real half; imag half); v [T, C] natural; weight
  panels pre-swizzled on host into the SBUF tile layout so every panel load
  is one contiguous DMA. Scores are computed as sT [tk, q] (softmax'd P
  feeds the AV matmul directly), four tk-blocks share one 2-bank PSUM tile
  so exp runs once per quad; softmax without max-subtraction; causal mask
  multiplicative on P post-exp; row-sums l via ones-column matmuls; 1/l
  broadcast via GPSIMD. qT/yT SBUF-resident. Writes go on the GPSIMD DMA
  queue, loads on the sync queue (avoids FIFO head-of-line blocking).
  Matmul operands bf16; accumulation f32.
"""

import math
import numpy as np
from contextlib import ExitStack

import ml_dtypes
import concourse.bass as bass
import concourse.tile as tile
from concourse import bacc, mybir
from concourse import bass_utils

F32 = mybir.dt.float32
BF16 = mybir.dt.bfloat16
AF = mybir.ActivationFunctionType
ALU = mybir.AluOpType

B, T, C, H, D = 4, 2048, 2048, 16, 128
HALF = D // 2
HO = H // 2                  # heads owned per core
CO = HO * D                  # 1024 own-head channels
S = 256                      # query stripe width
QL = 1024                    # query rows per core
NG = 4                       # stripe groups per core
NTK = [4, 8, 12, 16]         # unified tk-block count per group (128-wide blocks)
STRIPES = [[0, 3, 4, 7], [1, 2, 5, 6]]  # per parity: global stripe ids
SCALE = 1.0 / math.sqrt(D)
RG = [[0, 1], [2, 3], [4, 5], [6, 7]]
NCK = C // 128               # 16 contraction blocks

_NC_CACHE = None
TRACE = False
LAST_RESULTS = None


def _r3(ap, p=128):
    """[A*p, c] dram AP -> [p, A, c]."""
    return ap.rearrange("(a p) c -> p a c", p=p)


def build_nc():
    nc = bacc.Bacc("TRN2", target_bir_lowering=False, debug=False, num_devices=8)

    xT_d = nc.dram_tensor("xT", [C, T], BF16, kind="ExternalInput")
    xTq_d = nc.dram_tensor("xTq", [C, QL], BF16, kind="ExternalInput")
    # weight panels pre-swizzled to SBUF layout (one contiguous DMA each)
    wqp_d = nc.dram_tensor("wqp", [H, 128, NCK, 128], BF16, kind="ExternalInput")
    wkp_d = nc.dram_tensor("wkp", [HO, 128, NCK, 128], BF16, kind="ExternalInput")
    wvp_d = nc.dram_tensor("wvp", [2, 128, NCK, 512], BF16, kind="ExternalInput")
    wop_d = nc.dram_tensor("wop", [4, 128, H, 512], BF16, kind="ExternalInput")
    cosT_d = nc.dram_tensor("cosT", [D, T], BF16, kind="ExternalInput")
    sinT_d = nc.dram_tensor("sinT", [D, T], BF16, kind="ExternalInput")
    cosq_d = nc.dram_tensor("cosq", [D, QL], BF16, kind="ExternalInput")
    sinq_d = nc.dram_tensor("sinq", [D, QL], BF16, kind="ExternalInput")
    mask_d = nc.dram_tensor("mask01", [128, NG * 4 * S], BF16, kind="ExternalInput")
    ones_d = nc.dram_tensor("ones", [128, 128], BF16, kind="ExternalInput")
    out_d = nc.dram_tensor("out", [QL, C], F32, kind="ExternalOutput")

    k_own = nc.dram_tensor("k_own", [CO, T], BF16, kind="Internal")
    v_own = [nc.dram_tensor(f"v_own{i}", [T, CO // 2], BF16, kind="Internal")
             for i in range(2)]
    k_all = [nc.dram_tensor(f"k_all{i}", [2, CO // 2, T], BF16, kind="Internal")
             for i in range(2)]
    v_all = [nc.dram_tensor(f"v_all{i}", [2, T, CO // 2], BF16, kind="Internal")
             for i in range(2)]

    with tile.TileContext(nc) as tc:
        _build_body(nc, tc, xT_d, xTq_d, wqp_d, wkp_d, wvp_d, wop_d,
                    cosT_d, sinT_d, cosq_d, sinq_d, mask_d, ones_d, out_d,
                    k_own, v_own, k_all, v_all)
    nc.compile()
    return nc


def _rope(nc, rtmp, cp, ps, out_sb, cos, sin, w):
    """psum [128, w] f32 (rows 0:64 real, 64:128 imag) -> rope -> out_sb bf16.

    Copies psum to bf16 SBUF first (ACT) so all DVE ops run in 4x bf16 mode.
    """
    rr = cp.tile([128, w], BF16, tag="rr", name="rr")
    nc.scalar.copy(rr[:], ps[:])
    r, im = rr[0:HALF, :], rr[HALF:D, :]
    m1 = rtmp.tile([HALF, w], BF16, tag="m1", name="m1")
    m2 = rtmp.tile([HALF, w], BF16, tag="m2", name="m2")
    m3 = rtmp.tile([HALF, w], BF16, tag="m1", name="m3")
    m4 = rtmp.tile([HALF, w], BF16, tag="m2", name="m4")
    # cos/sin are partition-duplicated [128, w]; slices keep input bases equal
    nc.vector.tensor_mul(m1[:], r, cos[0:HALF, :])
    nc.vector.tensor_mul(m2[:], im, sin[HALF:D, :])
    nc.vector.tensor_sub(out_sb[0:HALF, :], m1[:], m2[:])
    nc.vector.tensor_mul(m3[:], r, sin[0:HALF, :])
    nc.vector.tensor_mul(m4[:], im, cos[HALF:D, :])
    nc.vector.tensor_add(out_sb[HALF:D, :], m3[:], m4[:])


def _build_body(nc, tc, xT_d, xTq_d, wqp_d, wkp_d, wvp_d, wop_d,
                cosT_d, sinT_d, cosq_d, sinq_d, mask_d, ones_d, out_d,
                k_own, v_own, k_all, v_all):
    with ExitStack() as top:
        qt_pool = top.enter_context(tc.tile_pool(name="qt", bufs=1))
        qT = qt_pool.tile([128, H, QL], BF16)
        kv = top.enter_context(tc.tile_pool(name="b_kv", bufs=2))
        mp = top.enter_context(tc.tile_pool(name="b_m", bufs=1))
        mk = mp.tile([128, NG * 4, S], BF16, tag="mask")
        ones = mp.tile([128, 128], BF16, tag="ones")
        state = {"loaded": -1}
        kv_t = {}

        def emit_loads(h):
            # per head-pair kT/v loads (issued on the gpsimd queue)
            hp = h // 2
            if hp * 2 > state["loaded"]:
                hi, hj = (hp * 2) // HO, (hp * 2) % HO
                hf, hr = hj // 4, hj % 4
                kT2 = kv.tile([128, 2, T], BF16, tag="kT", name="kT2")
                nc.sync.dma_start(
                    kT2[:], k_all[hf].ap()[hi, hr * 128:(hr + 2) * 128, :]
                    .rearrange("(b p) t -> p b t", p=128))
                vh2 = kv.tile([128, 16, 2, 128], BF16, tag="vh", name="vh2")
                nc.sync.dma_start(
                    vh2[:], _r3(v_all[hf].ap()[hi])[:, :, hr * 128:(hr + 2) * 128]
                    .rearrange("p a (b c) -> p a b c", b=2))
                kv_t[hp] = (kT2, vh2)
                state["loaded"] = hp * 2

        with ExitStack() as xctx:
            x_pool = xctx.enter_context(tc.tile_pool(name="ax", bufs=1))
            cs = xctx.enter_context(tc.tile_pool(name="a_cs", bufs=1))
            xT = x_pool.tile([128, NCK, T], BF16)
            xTq = x_pool.tile([128, NCK, QL], BF16)
            cosf = cs.tile([D, T], BF16, tag="cosf")
            sinf = cs.tile([D, T], BF16, tag="sinf")

            # -------- Phase A2+A3: own-head k (rope) and v projections --------
            with ExitStack() as ctx:
                wp = ctx.enter_context(tc.tile_pool(name="a2_w", bufs=2))
                wpv = ctx.enter_context(tc.tile_pool(name="a3_w", bufs=2))
                st = ctx.enter_context(tc.tile_pool(name="a2_st", bufs=2))
                cp = ctx.enter_context(tc.tile_pool(name="a2_cp", bufs=2))
                rtmp = ctx.enter_context(tc.tile_pool(name="a2_rt", bufs=1))
                psA = ctx.enter_context(tc.tile_pool(name="a2_ps", bufs=3, space="PSUM"))
                psV = ctx.enter_context(tc.tile_pool(name="a3_ps", bufs=3, space="PSUM"))

                # startup ordering: first weight panel + first xT chunk lead
                # the queue so the k-projection starts as early as possible
                wk_first = wp.tile([128, NCK, 128], BF16, tag="wk", name="wk_first")
                nc.sync.dma_start(wk_first[:], wkp_d.ap()[0])
                nc.sync.dma_start(xT[:, :, 0:512], _r3(xT_d.ap())[:, :, 0:512])
                nc.sync.dma_start(cosf[:], cosT_d.ap())
                nc.sync.dma_start(sinf[:], sinT_d.ap())
                for tg in range(1, 4):
                    nc.sync.dma_start(xT[:, :, tg * 512:(tg + 1) * 512],
                                      _r3(xT_d.ap())[:, :, tg * 512:(tg + 1) * 512])

                for hh in range(HO):
                    if hh == 0:
                        wk = wk_first
                    else:
                        wk = wp.tile([128, NCK, 128], BF16, tag="wk", name="wk")
                        nc.sync.dma_start(wk[:], wkp_d.ap()[hh])
                    for tg in range(4):
                        ps = psA.tile([128, 512], F32)
                        for ck in range(NCK):
                            nc.tensor.matmul(ps[:], wk[:, ck, :],
                                             xT[:, ck, tg * 512:(tg + 1) * 512],
                                             start=(ck == 0), stop=(ck == NCK - 1))
                        o = st.tile([128, 512], BF16, tag="o", name="o")
                        _rope(nc, rtmp, cp, ps, o,
                              cosf[:, tg * 512:(tg + 1) * 512],
                              sinf[:, tg * 512:(tg + 1) * 512], 512)
                        nc.sync.dma_start(
                            k_own.ap()[hh * 128:(hh + 1) * 128,
                                       tg * 512:(tg + 1) * 512], o[:])
                    if hh == 3 or hh == HO - 1:
                        half = hh // 4
                        nc.gpsimd.collective_compute(
                            "AllGather", ALU.bypass, replica_groups=RG,
                            ins=[k_own.ap()[half * 512:(half + 1) * 512, :]],
                            outs=[k_all[half].ap()])

                nc.sync.dma_start(xTq[:], _r3(xTq_d.ap()))
                for vg in range(4):  # own v column groups of 256
                    wv = wpv.tile([128, NCK, 256], BF16)
                    nc.sync.dma_start(
                        wv[:], wvp_d.ap().rearrange("a p c (g w) -> a p c g w", w=256)
                        [vg // 2, :, :, vg % 2, :])
                    for tb in range(16):
                        ps = psV.tile([128, 256], F32, name="psv")
                        for ck in range(NCK):
                            nc.tensor.matmul(ps[:], xT[:, ck, tb * 128:(tb + 1) * 128],
                                             wv[:, ck, :],
                                             start=(ck == 0), stop=(ck == NCK - 1))
                        o = st.tile([128, 256], BF16, tag="o2", name="ov")
                        nc.scalar.copy(o[:], ps[:])
                        nc.sync.dma_start(
                            v_own[vg // 2].ap()[tb * 128:(tb + 1) * 128,
                                                (vg % 2) * 256:(vg % 2 + 1) * 256],
                            o[:])
                    if vg % 2 == 1:
                        nc.gpsimd.collective_compute(
                            "AllGather", ALU.bypass, replica_groups=RG,
                            ins=[v_own[vg // 2].ap()], outs=[v_all[vg // 2].ap()])

            # ------------- Phase A1: q projection + rope -> qT (SBUF) -------------
            with ExitStack() as ctx:
                wp = ctx.enter_context(tc.tile_pool(name="a1_w", bufs=3))
                cs2 = ctx.enter_context(tc.tile_pool(name="a1_cs", bufs=1))
                cp = ctx.enter_context(tc.tile_pool(name="a1_cp", bufs=2))
                rtmp = ctx.enter_context(tc.tile_pool(name="a1_rt", bufs=3))
                psA = ctx.enter_context(tc.tile_pool(name="a1_ps", bufs=3, space="PSUM"))

                cosq = cs2.tile([D, QL], BF16, tag="cosq")
                sinq = cs2.tile([D, QL], BF16, tag="sinq")
                nc.sync.dma_start(cosq[:], cosq_d.ap())
                nc.sync.dma_start(sinq[:], sinq_d.ap())

                for h in range(H):
                    wq = wp.tile([128, NCK, 128], BF16)
                    nc.sync.dma_start(wq[:], wqp_d.ap()[h])
                    for qg in range(2):
                        ps = psA.tile([128, 512], F32)
                        for ck in range(NCK):
                            nc.tensor.matmul(ps[:], wq[:, ck, :],
                                             xTq[:, ck, qg * 512:(qg + 1) * 512],
                                             start=(ck == 0), stop=(ck == NCK - 1))
                        _rope(nc, rtmp, cp, ps, qT[:, h, qg * 512:(qg + 1) * 512],
                              cosq[:, qg * 512:(qg + 1) * 512],
                              sinq[:, qg * 512:(qg + 1) * 512], 512)

            # prefetch phase-B data (sync queue; AG waits block nothing here)
            nc.sync.dma_start(mk[:],
                              mask_d.ap().rearrange("p (a s) -> p a s", s=S))
            nc.sync.dma_start(ones[:], ones_d.ap())
            emit_loads(0)
            emit_loads(2)

        # ---------------- Phase B: attention -> yT (SBUF-resident) ----------------
        y_pool = top.enter_context(tc.tile_pool(name="b_y", bufs=1))
        yT = y_pool.tile([128, H, QL], BF16)
        wop = top.enter_context(tc.tile_pool(name="c_w", bufs=2))

        with ExitStack() as ctx:
            pp = ctx.enter_context(tc.tile_pool(name="b_p", bufs=3))
            lp = ctx.enter_context(tc.tile_pool(name="b_l", bufs=2))
            ps_s = ctx.enter_context(tc.tile_pool(name="b_ps_s", bufs=2, space="PSUM"))
            ps_y = ctx.enter_context(tc.tile_pool(name="b_ps_y", bufs=2, space="PSUM"))
            ps_l = ctx.enter_context(tc.tile_pool(name="b_ps_l", bufs=2, space="PSUM"))

            # flat software pipeline over every (h, g, quad) in phase B
            quads = [(h, g, qi) for h in range(H) for g in range(NG)
                     for qi in range(NTK[g] // 4)]
            sq_t = {}

            def emit_quad(idx):
                h, g, qi = quads[idx]
                emit_loads(h)
                kT2, _ = kv_t[h // 2]
                sq = ps_s.tile([128, 4 * S], F32, tag="sps", name="sps")
                for u in range(4):
                    j = qi * 4 + u
                    nc.tensor.matmul(sq[:, u * S:(u + 1) * S],
                                     kT2[:, h % 2, j * 128:(j + 1) * 128],
                                     qT[:, h, g * S:(g + 1) * S],
                                     start=True, stop=True)
                sq_t[idx] = sq

            PF = 2
            acc = {}
            for idx in range(min(PF, len(quads))):
                emit_quad(idx)
            for idx, (h, g, qi) in enumerate(quads):
                if idx + PF < len(quads):
                    emit_quad(idx + PF)
                sq = sq_t.pop(idx)
                ntk = NTK[g]
                nq = ntk // 4
                vh = kv_t[h // 2][1][:, :, h % 2, :]
                if qi == 0:
                    acc["yps"] = ps_y.tile([128, S], F32, name="yps")
                    acc["lps"] = ps_l.tile([128, S], F32, name="lps")
                yps, lps = acc["yps"], acc["lps"]
                p4 = pp.tile([128, 4 * S], BF16, tag="p", name="p4")
                nc.scalar.activation(p4[:], sq[:], AF.Exp, scale=SCALE)
                for u in range(4):
                    j = qi * 4 + u
                    jm = j - (ntk - 4)
                    if jm >= 0:
                        nc.vector.tensor_mul(
                            p4[:, u * S:(u + 1) * S],
                            p4[:, u * S:(u + 1) * S],
                            mk[:, g * 4 + jm, :])
                    nc.tensor.matmul(yps[:], vh[:, j, :],
                                     p4[:, u * S:(u + 1) * S],
                                     start=(j == 0), stop=(j == ntk - 1))
                    nc.tensor.matmul(lps[:], ones[:], p4[:, u * S:(u + 1) * S],
                                     start=(j == 0), stop=(j == ntk - 1))
                if qi == nq - 1:
                    # l rows are already broadcast across all partitions
                    rinv = lp.tile([128, S], F32, tag="rinv", name="rinv")
                    nc.vector.reciprocal(rinv[:], lps[:])
                    nc.vector.tensor_mul(yT[:, h, g * S:(g + 1) * S], yps[:],
                                         rinv[:])

        # ---------------- Phase C: output projection ----------------
        with ExitStack() as ctx:
            ost = ctx.enter_context(tc.tile_pool(name="c_o", bufs=3))
            psC = ctx.enter_context(tc.tile_pool(name="c_ps", bufs=2, space="PSUM"))

            for cg in range(4):
                wo = wop.tile([128, H, 512], BF16)
                nc.sync.dma_start(wo[:], wop_d.ap()[cg])
                for qb in range(8):
                    ps = psC.tile([128, 512], F32)
                    for h in range(H):
                        nc.tensor.matmul(ps[:], yT[:, h, qb * 128:(qb + 1) * 128],
                                         wo[:, h, :],
                                         start=(h == 0), stop=(h == H - 1))
                    o = ost.tile([128, 512], F32)
                    nc.scalar.copy(o[:], ps[:])
                    nc.gpsimd.dma_start(
                        out_d.ap()[qb * 128:(qb + 1) * 128, cg * 512:(cg + 1) * 512],
                        o[:])


# ------------------------- host-side wrapper -------------------------

def _rope_perm():
    p = np.empty(D, dtype=np.int64)
    p[0:HALF] = np.arange(0, D, 2)
    p[HALF:D] = np.arange(1, D, 2)
    return p


def _panelize(w, npanel, pw):
    """[C, npanel*pw] -> [npanel, 128, NCK, pw] (SBUF tile layout)."""
    return np.ascontiguousarray(
        w.reshape(NCK, 128, npanel, pw).transpose(2, 1, 0, 3))


def _make_mask(s):
    """Multiplicative post-exp mask: 1 = valid (tk <= q position), 0 = masked."""
    mk = np.zeros((128, NG * 4, S), np.float32)
    tk = np.arange(128)[:, None]
    qq = np.arange(S)[None, :]
    for g in range(NG):
        q0 = STRIPES[s][g] * S
        for jj in range(4):
            tk0 = (NTK[g] - 4 + jj) * 128
            mk[:, g * 4 + jj, :] = ((tk0 + tk) <= (q0 + qq)).astype(np.float32)
    return mk.reshape(128, NG * 4 * S)


def kernel(x, wqkv, wo, rope_cos, rope_sin):
    global _NC_CACHE, LAST_RESULTS
    x = np.asarray(x, dtype=np.float32)
    wqkv = np.asarray(wqkv, dtype=np.float32)
    wo = np.asarray(wo, dtype=np.float32)
    rope_cos = np.asarray(rope_cos, dtype=np.float32)
    rope_sin = np.asarray(rope_sin, dtype=np.float32)

    if _NC_CACHE is None:
        _NC_CACHE = build_nc()
    nc = _NC_CACHE

    bf = ml_dtypes.bfloat16
    perm = _rope_perm()
    wq = wqkv[:, 0:C].reshape(C, H, D)[:, :, perm].reshape(C, C)
    wk = wqkv[:, C:2 * C].reshape(C, H, D)[:, :, perm].reshape(C, C)
    wv = wqkv[:, 2 * C:]
    wqp = _panelize(wq.astype(bf), H, 128)
    wkp = [_panelize(wk[:, s * CO:(s + 1) * CO].astype(bf), HO, 128)
           for s in range(2)]
    wvp = [_panelize(wv[:, s * CO:(s + 1) * CO].astype(bf), 2, 512)
           for s in range(2)]
    # wop: [4, 128, H, 512]: element [cg, p, h, c] = wo[h*128 + p, cg*512 + c]
    wop = np.ascontiguousarray(
        wo.astype(bf).reshape(H, 128, 4, 512).transpose(2, 1, 0, 3))
    cosT = np.ascontiguousarray(np.vstack([rope_cos.T, rope_cos.T])).astype(bf)
    sinT = np.ascontiguousarray(np.vstack([rope_sin.T, rope_sin.T])).astype(bf)
    ones = np.ones((128, 128), bf)
    masks = [_make_mask(0).astype(bf), _make_mask(1).astype(bf)]
    qrows = [np.concatenate([np.arange(st * S, (st + 1) * S) for st in STRIPES[s]])
             for s in range(2)]

    in_maps = []
    for c in range(8):
        b, s = c // 2, c % 2
        in_maps.append({
            "xT": np.ascontiguousarray(x[b].T).astype(bf),
            "xTq": np.ascontiguousarray(x[b][qrows[s]].T).astype(bf),
            "wqp": wqp,
            "wkp": wkp[s],
            "wvp": wvp[s],
            "wop": wop,
            "cosT": cosT,
            "sinT": sinT,
            "cosq": np.ascontiguousarray(
                np.vstack([rope_cos[qrows[s]].T] * 2)).astype(bf),
            "sinq": np.ascontiguousarray(
                np.vstack([rope_sin[qrows[s]].T] * 2)).astype(bf),
            "mask01": masks[s],
            "ones": ones,
        })

    res = bass_utils.run_bass_kernel_spmd(nc, in_maps, core_ids=list(range(8)),
                                          trace=TRACE,
                                          trace_cores=list(range(8)) if TRACE else None,
                                          stitch_traces=False)
    LAST_RESULTS = res

    out = np.empty((B, T, C), np.float32)
    for c in range(8):
        b, s = c // 2, c % 2
        out[b][qrows[s]] = res.results[c]["out"]
    return out


# revision 35
# speedup vs baseline: 1.1942x; 1.1942x over previous
"""Causal self-attention (B=4, T=2048, C=2048, H=16) on 8 TRN2 NeuronCores.

Sharding: core c owns batch b=c//2 and a balanced set of query stripes
(parity s=c%2). KV projection is split by head between the two cores of a
batch (even core: heads 0-7, odd core: heads 8-15) and exchanged with a
pairwise AllGather — rank order makes gathered head-slot order identical on
both cores, so the SPMD graph stays symmetric. Attention + output projection
run on each core's 1024 query rows; the host gathers the row-shards.

Layouts (all transposed so no on-chip transposes are ever needed):
  xT [C, T] (host-transposed); qT/kT [C_head, T*] head-major with
  rope-paired-permuted d (real half; imag half); v [T, C] natural; weight
  panels pre-swizzled on host into the SBUF tile layout so every panel load
  is one contiguous DMA. Scores are computed as sT [tk, q] (softmax'd P
  feeds the AV matmul directly), four tk-blocks share one 2-bank PSUM tile
  so exp runs once per quad; softmax without max-subtraction; causal mask
  multiplicative on P post-exp; row-sums l via ones-column matmuls; 1/l
  broadcast via GPSIMD. qT/yT SBUF-resident. Writes go on the GPSIMD DMA
  queue, loads on the sync queue (avoids FIFO head-of-line blocking).
  Matmul operands bf16; accumulation f32.
"""

import math
import numpy as np
from contextlib import ExitStack

import ml_dtypes
import concourse.bass as bass
import concourse.tile as tile
from concourse import bacc, mybir
from concourse import bass_utils

F32 = mybir.dt.float32
BF16 = mybir.dt.bfloat16
AF = mybir.ActivationFunctionType
ALU = mybir.AluOpType

B, T, C, H, D = 4, 2048, 2048, 16, 128
HALF = D // 2
HO = H // 2                  # heads owned per core
CO = HO * D                  # 1024 own-head channels
S = 256                      # query stripe width
QL = 1024                    # query rows per core
NG = 4                       # stripe groups per core
NTK = [4, 8, 12, 16]         # unified tk-block count per group (128-wide blocks)
STRIPES = [[0, 3, 4, 7], [1, 2, 5, 6]]  # per parity: global stripe ids
SCALE = 1.0 / math.sqrt(D)
RG = [[0, 1], [2, 3], [4, 5], [6, 7]]
NCK = C // 128               # 16 contraction blocks

_NC_CACHE = None
TRACE = False
LAST_RESULTS = None


def _r3(ap, p=128):
    """[A*p, c] dram AP -> [p, A, c]."""
    return ap.rearrange("(a p) c -> p a c", p=p)


def build_nc():
    nc = bacc.Bacc("TRN2", target_bir_lowering=False, debug=False, num_devices=8)

    xT_d = nc.dram_tensor("xT", [C, T], BF16, kind="ExternalInput")
    xTq_d = nc.dram_tensor("xTq", [C, QL], BF16, kind="ExternalInput")
    # weight panels pre-swizzled to SBUF layout (one contiguous DMA each)
    wqp_d = nc.dram_tensor("wqp", [H, 128, NCK, 128], BF16, kind="ExternalInput")
    wkp_d = nc.dram_tensor("wkp", [HO, 128, NCK, 128], BF16, kind="ExternalInput")
    wvp_d = nc.dram_tensor("wvp", [2, 128, NCK, 512], BF16, kind="ExternalInput")
    wop_d = nc.dram_tensor("wop", [4, 128, H, 512], BF16, kind="ExternalInput")
    cosT_d = nc.dram_tensor("cosT", [D, T], BF16, kind="ExternalInput")
    sinT_d = nc.dram_tensor("sinT", [D, T], BF16, kind="ExternalInput")
    cosq_d = nc.dram_tensor("cosq", [D, QL], BF16, kind="ExternalInput")
    sinq_d = nc.dram_tensor("sinq", [D, QL], BF16, kind="ExternalInput")
    mask_d = nc.dram_tensor("mask01", [128, NG * 4 * S], BF16, kind="ExternalInput")
    ones_d = nc.dram_tensor("ones", [128, 128], BF16, kind="ExternalInput")
    out_d = nc.dram_tensor("out", [QL, C], F32, kind="ExternalOutput")

    k_own = nc.dram_tensor("k_own", [CO, T], BF16, kind="Internal")
    v_own = [nc.dram_tensor(f"v_own{i}", [T, CO // 2], BF16, kind="Internal")
             for i in range(2)]
    k_all = [nc.dram_tensor(f"k_all{i}", [2, CO // 2, T], BF16, kind="Internal")
             for i in range(2)]
    v_all = [nc.dram_tensor(f"v_all{i}", [2, T, CO // 2], BF16, kind="Internal")
             for i in range(2)]

    with tile.TileContext(nc) as tc:
        _build_body(nc, tc, xT_d, xTq_d, wqp_d, wkp_d, wvp_d, wop_d,
                    cosT_d, sinT_d, cosq_d, sinq_d, mask_d, ones_d, out_d,
                    k_own, v_own, k_all, v_all)
    nc.compile()
    return nc


def _rope(nc, rtmp, cp, ps, out_sb, cos, sin, w):
    """psum [128, w] f32 (rows 0:64 real, 64:128 imag) -> rope -> out_sb bf16.

    Copies psum to bf16 SBUF first (ACT) so all DVE ops run in 4x bf16 mode.
    """
    rr = cp.tile([128, w], BF16, tag="rr", name="rr")
    nc.scalar.copy(rr[:], ps[:])
    r, im = rr[0:HALF, :], rr[HALF:D, :]
    m1 = rtmp.tile([HALF, w], BF16, tag="m1", name="m1")
    m2 = rtmp.tile([HALF, w], BF16, tag="m2", name="m2")
    m3 = rtmp.tile([HALF, w], BF16, tag="m1", name="m3")
    m4 = rtmp.tile([HALF, w], BF16, tag="m2", name="m4")
    # cos/sin are partition-duplicated [128, w]; slices keep input bases equal
    nc.vector.tensor_mul(m1[:], r, cos[0:HALF, :])
    nc.vector.tensor_mul(m2[:], im, sin[HALF:D, :])
    nc.vector.tensor_sub(out_sb[0:HALF, :], m1[:], m2[:])
    nc.vector.tensor_mul(m3[:], r, sin[0:HALF, :])
    nc.vector.tensor_mul(m4[:], im, cos[HALF:D, :])
    nc.vector.tensor_add(out_sb[HALF:D, :], m3[:], m4[:])


def _build_body(nc, tc, xT_d, xTq_d, wqp_d, wkp_d, wvp_d, wop_d,
                cosT_d, sinT_d, cosq_d, sinq_d, mask_d, ones_d, out_d,
                k_own, v_own, k_all, v_all):
    with ExitStack() as top:
        qt_pool = top.enter_context(tc.tile_pool(name="qt", bufs=1))
        qT = qt_pool.tile([128, H, QL], BF16)
        kv = top.enter_context(tc.tile_pool(name="b_kv", bufs=2))
        mp = top.enter_context(tc.tile_pool(name="b_m", bufs=1))
        mk = mp.tile([128, NG * 4, S], BF16, tag="mask")
        ones = mp.tile([128, 128], BF16, tag="ones")
        state = {"loaded": -1}
        kv_t = {}

        def emit_loads(h):
            # per head-pair kT/v loads (issued on the gpsimd queue)
            hp = h // 2
            if hp * 2 > state["loaded"]:
                hi, hj = (hp * 2) // HO, (hp * 2) % HO
                hf, hr = hj // 4, hj % 4
                kT2 = kv.tile([128, 2, T], BF16, tag="kT", name="kT2")
                nc.sync.dma_start(
                    kT2[:], k_all[hf].ap()[hi, hr * 128:(hr + 2) * 128, :]
                    .rearrange("(b p) t -> p b t", p=128))
                vh2 = kv.tile([128, 16, 2, 128], BF16, tag="vh", name="vh2")
                nc.sync.dma_start(
                    vh2[:], _r3(v_all[hf].ap()[hi])[:, :, hr * 128:(hr + 2) * 128]
                    .rearrange("p a (b c) -> p a b c", b=2))
                kv_t[hp] = (kT2, vh2)
                state["loaded"] = hp * 2

        with ExitStack() as xctx:
            x_pool = xctx.enter_context(tc.tile_pool(name="ax", bufs=1))
            cs = xctx.enter_context(tc.tile_pool(name="a_cs", bufs=1))
            xT = x_pool.tile([128, NCK, T], BF16)
            xTq = x_pool.tile([128, NCK, QL], BF16)
            cosf = cs.tile([D, T], BF16, tag="cosf")
            sinf = cs.tile([D, T], BF16, tag="sinf")

            # -------- Phase A2+A3: own-head k (rope) and v projections --------
            with ExitStack() as ctx:
                wp = ctx.enter_context(tc.tile_pool(name="a2_w", bufs=2))
                wpv = ctx.enter_context(tc.tile_pool(name="a3_w", bufs=2))
                st = ctx.enter_context(tc.tile_pool(name="a2_st", bufs=2))
                cp = ctx.enter_context(tc.tile_pool(name="a2_cp", bufs=2))
                rtmp = ctx.enter_context(tc.tile_pool(name="a2_rt", bufs=1))
                psA = ctx.enter_context(tc.tile_pool(name="a2_ps", bufs=3, space="PSUM"))
                psV = ctx.enter_context(tc.tile_pool(name="a3_ps", bufs=3, space="PSUM"))

                # startup ordering: first weight panel + first xT chunk lead
                # the queue so the k-projection starts as early as possible
                wk_first = wp.tile([128, NCK, 128], BF16, tag="wk", name="wk_first")
                nc.sync.dma_start(wk_first[:], wkp_d.ap()[0])
                nc.sync.dma_start(xT[:, :, 0:512], _r3(xT_d.ap())[:, :, 0:512])
                nc.sync.dma_start(cosf[:], cosT_d.ap())
                nc.sync.dma_start(sinf[:], sinT_d.ap())
                for tg in range(1, 4):
                    nc.sync.dma_start(xT[:, :, tg * 512:(tg + 1) * 512],
                                      _r3(xT_d.ap())[:, :, tg * 512:(tg + 1) * 512])

                for hh in range(HO):
                    if hh == 0:
                        wk = wk_first
                    else:
                        wk = wp.tile([128, NCK, 128], BF16, tag="wk", name="wk")
                        nc.sync.dma_start(wk[:], wkp_d.ap()[hh])
                    for tg in range(4):
                        ps = psA.tile([128, 512], F32)
                        for ck in range(NCK):
                            nc.tensor.matmul(ps[:], wk[:, ck, :],
                                             xT[:, ck, tg * 512:(tg + 1) * 512],
                                             start=(ck == 0), stop=(ck == NCK - 1))
                        o = st.tile([128, 512], BF16, tag="o", name="o")
                        _rope(nc, rtmp, cp, ps, o,
                              cosf[:, tg * 512:(tg + 1) * 512],
                              sinf[:, tg * 512:(tg + 1) * 512], 512)
                        nc.sync.dma_start(
                            k_own.ap()[hh * 128:(hh + 1) * 128,
                                       tg * 512:(tg + 1) * 512], o[:])
                    if hh == 3 or hh == HO - 1:
                        half = hh // 4
                        nc.gpsimd.collective_compute(
                            "AllGather", ALU.bypass, replica_groups=RG,
                            ins=[k_own.ap()[half * 512:(half + 1) * 512, :]],
                            outs=[k_all[half].ap()])

                nc.sync.dma_start(xTq[:], _r3(xTq_d.ap()))
                for vg in range(4):  # own v column groups of 256
                    wv = wpv.tile([128, NCK, 256], BF16)
                    nc.sync.dma_start(
                        wv[:], wvp_d.ap().rearrange("a p c (g w) -> a p c g w", w=256)
                        [vg // 2, :, :, vg % 2, :])
                    for tb in range(16):
                        ps = psV.tile([128, 256], F32, name="psv")
                        for ck in range(NCK):
                            nc.tensor.matmul(ps[:], xT[:, ck, tb * 128:(tb + 1) * 128],
                                             wv[:, ck, :],
                                             start=(ck == 0), stop=(ck == NCK - 1))
                        o = st.tile([128, 256], BF16, tag="o2", name="ov")
                        nc.scalar.copy(o[:], ps[:])
                        nc.sync.dma_start(
                            v_own[vg // 2].ap()[tb * 128:(tb + 1) * 128,
                                                (vg % 2) * 256:(vg % 2 + 1) * 256],
                            o[:])
                    if vg % 2 == 1:
                        nc.gpsimd.collective_compute(
                            "AllGather", ALU.bypass, replica_groups=RG,
                            ins=[v_own[vg // 2].ap()], outs=[v_all[vg // 2].ap()])

            # ------------- Phase A1: q projection + rope -> qT (SBUF) -------------
            with ExitStack() as ctx:
                wp = ctx.enter_context(tc.tile_pool(name="a1_w", bufs=3))
                cs2 = ctx.enter_context(tc.tile_pool(name="a1_cs", bufs=1))
                cp = ctx.enter_context(tc.tile_pool(name="a1_cp", bufs=2))
                rtmp = ctx.enter_context(tc.tile_pool(name="a1_rt", bufs=3))
                psA = ctx.enter_context(tc.tile_pool(name="a1_ps", bufs=3, space="PSUM"))

                cosq = cs2.tile([D, QL], BF16, tag="cosq")
                sinq = cs2.tile([D, QL], BF16, tag="sinq")
                nc.sync.dma_start(cosq[:], cosq_d.ap())
                nc.sync.dma_start(sinq[:], sinq_d.ap())

                for h in range(H):
                    wq = wp.tile([128, NCK, 128], BF16)
                    nc.sync.dma_start(wq[:], wqp_d.ap()[h])
                    for qg in range(2):
                        ps = psA.tile([128, 512], F32)
                        for ck in range(NCK):
                            nc.tensor.matmul(ps[:], wq[:, ck, :],
                                             xTq[:, ck, qg * 512:(qg + 1) * 512],
                                             start=(ck == 0), stop=(ck == NCK - 1))
                        _rope(nc, rtmp, cp, ps, qT[:, h, qg * 512:(qg + 1) * 512],
                              cosq[:, qg * 512:(qg + 1) * 512],
                              sinq[:, qg * 512:(qg + 1) * 512], 512)

            # prefetch phase-B data (sync queue; AG waits block nothing here)
            nc.sync.dma_start(mk[:],
                              mask_d.ap().rearrange("p (a s) -> p a s", s=S))
            nc.sync.dma_start(ones[:], ones_d.ap())
            emit_loads(0)
            emit_loads(2)

        # ---------------- Phase B: attention -> yT (SBUF-resident) ----------------
        y_pool = top.enter_context(tc.tile_pool(name="b_y", bufs=1))
        yT = y_pool.tile([128, H, QL], BF16)
        wop = top.enter_context(tc.tile_pool(name="c_w", bufs=2))

        with ExitStack() as ctx:
            pp = ctx.enter_context(tc.tile_pool(name="b_p", bufs=3))
            lp = ctx.enter_context(tc.tile_pool(name="b_l", bufs=2))
            ps_s = ctx.enter_context(tc.tile_pool(name="b_ps_s", bufs=2, space="PSUM"))
            ps_y = ctx.enter_context(tc.tile_pool(name="b_ps_y", bufs=2, space="PSUM"))
            ps_l = ctx.enter_context(tc.tile_pool(name="b_ps_l", bufs=2, space="PSUM"))

            # flat software pipeline over every (h, g, quad) in phase B
            quads = [(h, g, qi) for h in range(H) for g in range(NG)
                     for qi in range(NTK[g] // 4)]
            sq_t = {}

            def emit_quad(idx):
                h, g, qi = quads[idx]
                emit_loads(h)
                kT2, _ = kv_t[h // 2]
                sq = ps_s.tile([128, 4 * S], F32, tag="sps", name="sps")
                for u in range(4):
                    j = qi * 4 + u
                    nc.tensor.matmul(sq[:, u * S:(u + 1) * S],
                                     kT2[:, h % 2, j * 128:(j + 1) * 128],
                                     qT[:, h, g * S:(g + 1) * S],
                                     start=True, stop=True)
                sq_t[idx] = sq

            PF = 2
            acc = {}
            for idx in range(min(PF, len(quads))):
                emit_quad(idx)
            for idx, (h, g, qi) in enumerate(quads):
                if idx + PF < len(quads):
                    emit_quad(idx + PF)
                sq = sq_t.pop(idx)
                ntk = NTK[g]
                nq = ntk // 4
                vh = kv_t[h // 2][1][:, :, h % 2, :]
                if qi == 0:
                    acc["yps"] = ps_y.tile([128, S], F32, name="yps")
                    acc["lps"] = ps_l.tile([128, S], F32, name="lps")
                yps, lps = acc["yps"], acc["lps"]
                p4 = pp.tile([128, 4 * S], BF16, tag="p", name="p4")
                nc.scalar.activation(p4[:], sq[:], AF.Exp, scale=SCALE)
                for u in range(4):
                    j = qi * 4 + u
                    jm = j - (ntk - 4)
                    if jm >= 0:
                        nc.vector.tensor_mul(
                            p4[:, u * S:(u + 1) * S],
                            p4[:, u * S:(u + 1) * S],
                            mk[:, g * 4 + jm, :])
                    nc.tensor.matmul(yps[:], vh[:, j, :],
                                     p4[:, u * S:(u + 1) * S],
                                     start=(j == 0), stop=(j == ntk - 1))
                # pre-sum P pairs on DVE so l needs half the matmuls
                lt = lp.tile([128, 2 * S], BF16, tag="lt", name="lt")
                nc.vector.tensor_add(lt[:, 0:S], p4[:, 0:S], p4[:, S:2 * S])
                nc.vector.tensor_add(lt[:, S:2 * S], p4[:, 2 * S:3 * S],
                                     p4[:, 3 * S:4 * S])
                nc.tensor.matmul(lps[:], ones[:], lt[:, 0:S],
                                 start=(qi == 0), stop=False)
                nc.tensor.matmul(lps[:], ones[:], lt[:, S:2 * S],
                                 start=False, stop=(qi == nq - 1))
                if qi == nq - 1:
                    # l rows are already broadcast across all partitions
                    rinv = lp.tile([128, S], F32, tag="rinv", name="rinv")
                    scr = lp.tile([128, S], F32, tag="scr", name="scr")
                    lsb = lp.tile([128, S], F32, tag="lsb2", name="lsb")
                    nc.vector.tensor_copy(lsb[:], lps[:])
                    nc.vector.reciprocal_approx_fast(rinv[:], lsb[:])
                    nc.vector.tensor_mul(yT[:, h, g * S:(g + 1) * S], yps[:],
                                         rinv[:])

        # ---------------- Phase C: output projection ----------------
        with ExitStack() as ctx:
            ost = ctx.enter_context(tc.tile_pool(name="c_o", bufs=3))
            psC = ctx.enter_context(tc.tile_pool(name="c_ps", bufs=2, space="PSUM"))

            for cg in range(4):
                wo = wop.tile([128, H, 512], BF16)
                nc.sync.dma_start(wo[:], wop_d.ap()[cg])
                for qb in range(8):
                    ps = psC.tile([128, 512], F32)
                    for h in range(H):
                        nc.tensor.matmul(ps[:], yT[:, h, qb * 128:(qb + 1) * 128],
                                         wo[:, h, :],
                                         start=(h == 0), stop=(h == H - 1))
                    o = ost.tile([128, 512], F32)
                    nc.scalar.copy(o[:], ps[:])
                    nc.gpsimd.dma_start(
                        out_d.ap()[qb * 128:(qb + 1) * 128, cg * 512:(cg + 1) * 512],
                        o[:])


# ------------------------- host-side wrapper -------------------------

def _rope_perm():
    p = np.empty(D, dtype=np.int64)
    p[0:HALF] = np.arange(0, D, 2)
    p[HALF:D] = np.arange(1, D, 2)
    return p


def _panelize(w, npanel, pw):
    """[C, npanel*pw] -> [npanel, 128, NCK, pw] (SBUF tile layout)."""
    return np.ascontiguousarray(
        w.reshape(NCK, 128, npanel, pw).transpose(2, 1, 0, 3))


def _make_mask(s):
    """Multiplicative post-exp mask: 1 = valid (tk <= q position), 0 = masked."""
    mk = np.zeros((128, NG * 4, S), np.float32)
    tk = np.arange(128)[:, None]
    qq = np.arange(S)[None, :]
    for g in range(NG):
        q0 = STRIPES[s][g] * S
        for jj in range(4):
            tk0 = (NTK[g] - 4 + jj) * 128
            mk[:, g * 4 + jj, :] = ((tk0 + tk) <= (q0 + qq)).astype(np.float32)
    return mk.reshape(128, NG * 4 * S)


def kernel(x, wqkv, wo, rope_cos, rope_sin):
    global _NC_CACHE, LAST_RESULTS
    x = np.asarray(x, dtype=np.float32)
    wqkv = np.asarray(wqkv, dtype=np.float32)
    wo = np.asarray(wo, dtype=np.float32)
    rope_cos = np.asarray(rope_cos, dtype=np.float32)
    rope_sin = np.asarray(rope_sin, dtype=np.float32)

    if _NC_CACHE is None:
        _NC_CACHE = build_nc()
    nc = _NC_CACHE

    bf = ml_dtypes.bfloat16
    perm = _rope_perm()
    wq = wqkv[:, 0:C].reshape(C, H, D)[:, :, perm].reshape(C, C)
    wk = wqkv[:, C:2 * C].reshape(C, H, D)[:, :, perm].reshape(C, C)
    wv = wqkv[:, 2 * C:]
    wqp = _panelize(wq.astype(bf), H, 128)
    wkp = [_panelize(wk[:, s * CO:(s + 1) * CO].astype(bf), HO, 128)
           for s in range(2)]
    wvp = [_panelize(wv[:, s * CO:(s + 1) * CO].astype(bf), 2, 512)
           for s in range(2)]
    # wop: [4, 128, H, 512]: element [cg, p, h, c] = wo[h*128 + p, cg*512 + c]
    wop = np.ascontiguousarray(
        wo.astype(bf).reshape(H, 128, 4, 512).transpose(2, 1, 0, 3))
    cosT = np.ascontiguousarray(np.vstack([rope_cos.T, rope_cos.T])).astype(bf)
    sinT = np.ascontiguousarray(np.vstack([rope_sin.T, rope_sin.T])).astype(bf)
    ones = np.ones((128, 128), bf)
    masks = [_make_mask(0).astype(bf), _make_mask(1).astype(bf)]
    qrows = [np.concatenate([np.arange(st * S, (st + 1) * S) for st in STRIPES[s]])
             for s in range(2)]

    in_maps = []
    for c in range(8):
        b, s = c // 2, c % 2
        in_maps.append({
            "xT": np.ascontiguousarray(x[b].T).astype(bf),
            "xTq": np.ascontiguousarray(x[b][qrows[s]].T).astype(bf),
            "wqp": wqp,
            "wkp": wkp[s],
            "wvp": wvp[s],
            "wop": wop,
            "cosT": cosT,
            "sinT": sinT,
            "cosq": np.ascontiguousarray(
                np.vstack([rope_cos[qrows[s]].T] * 2)).astype(bf),
            "sinq": np.ascontiguousarray(
                np.vstack([rope_sin[qrows[s]].T] * 2)).astype(bf),
            "mask01": masks[s],
            "ones": ones,
        })

    res = bass_utils.run_bass_kernel_spmd(nc, in_maps, core_ids=list(range(8)),
                                          trace=TRACE,
                                          trace_cores=list(range(8)) if TRACE else None,
                                          stitch_traces=False)
    LAST_RESULTS = res

    out = np.empty((B, T, C), np.float32)
    for c in range(8):
        b, s = c // 2, c % 2
        out[b][qrows[s]] = res.results[c]["out"]
    return out
